# revision 14
# baseline (speedup 1.0000x reference)
"""Trainium2 Bass kernel for ComplexConv4dNet (4-layer 4D CNN + training-mode BN).

Sharding: 8 cores = N(2) x D1-quarters(4, 3 slices each).
Per core all activations live in SBUF, padded layout [C, 5, 14, 14, 14]
(d1: 3 owned + 2 halo; d2/d3/d4: 12 + 1 zero-pad each side).
Each conv tap = accumulating PE matmul over a shifted window view.
BN stats: bn_stats on psum chunks -> AllReduce of (mean/8, E[x^2]/8).
Halos: L1 computes a 1-slice margin redundantly (no exchange); h2/h3 halos go
over a bf16 slab AllGather (groups of 4 same-n cores) + indirect-DMA gather,
with edge cores masking their out-of-domain halo slices to zero.

Host/dispatch layer: one AOT-compiled shard_map executable cached across
calls; weights + per-core constants stay device-resident; the only per-call
upload is the compact padded x shard (im2col happens on device via 27 DMAs);
output travels back as bf16.
"""

import ml_dtypes
import numpy as np

import concourse.bass as bass
import concourse.mybir as mybir
import concourse.tile as tile
from concourse import bacc
from concourse.bass import IndirectOffsetOnAxis

N_CORES = 8
D = 12
EPS = 1e-5
F32 = mybir.dt.float32
BF16 = mybir.dt.bfloat16
I32 = mybir.dt.int32
AF = mybir.ActivationFunctionType
ALU = mybir.AluOpType

# chunking: free chunk = (d1 slice, group of 3 d2 rows) -> [3,12,12] = 432
N_D2G = 4


def ff(ap):
    """Flatten the free (non-partition) dims of an AP."""
    n = len(ap.shape) - 1
    names = " ".join(f"d{i}" for i in range(n))
    return ap.rearrange(f"p {names} -> p ({names})")


def _build_module():
    nc = bacc.Bacc(None, target_bir_lowering=False)

    # ---- kernel I/O ----
    # xps: padded x shard, d1 slices [3q .. 3q+7) of the (2,2)-padded d1 axis,
    # full 14^3 padded e/f/g. The 27-way im2col expansion happens on device.
    xps = nc.dram_tensor("xps", [1, 7, 14, 14, 14], BF16, kind="ExternalInput")
    w1 = nc.dram_tensor("w1t", [27, 3, 64], BF16, kind="ExternalInput")
    w2a = nc.dram_tensor("w2a", [128, 81, 128], BF16, kind="ExternalInput")
    w3t = nc.dram_tensor("w3t", [128, 81, 64], BF16, kind="ExternalInput")
    w4t = nc.dram_tensor("w4t", [128, 81], BF16, kind="ExternalInput")
    g1 = nc.dram_tensor("g1", [64, 1], F32, kind="ExternalInput")
    be1 = nc.dram_tensor("be1", [64, 1], F32, kind="ExternalInput")
    g2 = nc.dram_tensor("g2", [128, 1], F32, kind="ExternalInput")
    be2 = nc.dram_tensor("be2", [128, 1], F32, kind="ExternalInput")
    g3 = nc.dram_tensor("g3", [64, 1], F32, kind="ExternalInput")
    be3 = nc.dram_tensor("be3", [64, 1], F32, kind="ExternalInput")
    b4 = nc.dram_tensor("b4", [1, 1], F32, kind="ExternalInput")
    ml = nc.dram_tensor("ml", [1, 1], F32, kind="ExternalInput")  # 0 if q==0
    mr = nc.dram_tensor("mr", [1, 1], F32, kind="ExternalInput")  # 0 if q==3
    hidx128 = nc.dram_tensor("hidx128", [128, 2], I32, kind="ExternalInput")
    hidx64 = nc.dram_tensor("hidx64", [64, 2], I32, kind="ExternalInput")
    yout = nc.dram_tensor("yout", [1, 3, 4, 3, 12, 12], BF16, kind="ExternalOutput")

    RG_ALL = [list(range(N_CORES))]
    RG_N = [[0, 1, 2, 3], [4, 5, 6, 7]]

    with tile.TileContext(nc) as tc:
        with (
            tc.tile_pool(name="consts", bufs=1) as consts,
            tc.tile_pool(name="hbig", bufs=2) as hbig,
            tc.tile_pool(name="wpool", bufs=1) as wpool,
            tc.tile_pool(name="psum", bufs=6, space="PSUM") as psum,
            tc.tile_pool(name="stats", bufs=1) as stats,
            tc.tile_pool(name="slabs", bufs=1) as slabs,
            tc.tile_pool(name="small", bufs=2) as small,
            tc.tile_pool(name="dram", bufs=1, space="DRAM") as dram,
        ):
            # ---- load constants + device-side im2col of the x shard ----
            # xc[t3] = xps[0, dd:dd+5, de:de+12, df:df+12, :] for t3=(dd,de,df)
            xc = hbig.tile([27, 5, 12, 12, 14], BF16, tag="h")
            im2col_engines = [nc.sync, nc.scalar, nc.gpsimd]
            for t3 in range(27):
                dd, de, df = t3 // 9, (t3 // 3) % 3, t3 % 3
                im2col_engines[t3 % 3].dma_start(
                    xc[t3:t3 + 1],
                    xps[0:1, dd:dd + 5, de:de + 12, df:df + 12, :],
                )
            w1sb = consts.tile([27, 3, 64], BF16)
            nc.sync.dma_start(w1sb[:], w1[:])
            w2asb = wpool.tile([128, 81, 128], BF16, tag="wa")
            nc.sync.dma_start(w2asb[:], w2a[:])

            def bc_load(handle, p):
                t = consts.tile([p, 1], F32, tag=f"bc_{handle.name}_{p}")
                nc.sync.dma_start(t[:], handle.ap().to_broadcast([p, 1]))
                return t

            g1sb, be1sb = bc_load(g1, 64), bc_load(be1, 64)
            g2sb, be2sb = bc_load(g2, 128), bc_load(be2, 128)
            g3sb, be3sb = bc_load(g3, 64), bc_load(be3, 64)
            b4sb = bc_load(b4, 1)
            ml64, mr64 = bc_load(ml, 64), bc_load(mr, 64)
            ml128, mr128 = bc_load(ml, 128), bc_load(mr, 128)
            hix128 = consts.tile([128, 2], I32)
            nc.sync.dma_start(hix128[:], hidx128[:])
            hix64 = consts.tile([64, 2], I32)
            nc.sync.dma_start(hix64[:], hidx64[:])

            eps64 = consts.tile([64, 1], F32)
            nc.vector.memset(eps64[:], EPS)
            eps128 = consts.tile([128, 1], F32)
            nc.vector.memset(eps128[:], EPS)

            # -------- helpers --------
            def stats_to_AB(mv, C, gamma, beta, epst, rg, name):
                """mv [C,2] = (mean, var) over the local 5184 owned voxels.
                AllReduce (mean/8, E[x^2]/8) -> global (A, B) with
                A = gamma * rsqrt(var + eps), B = beta - mean * A."""
                sq = small.tile([C, 1], F32, tag=f"sq{name}")
                nc.vector.tensor_mul(sq[:], mv[:, 0:1], mv[:, 0:1])
                arin_sb = small.tile([C, 2], F32, tag=f"arin{name}")
                # arin[:,0] = mean/8 ; arin[:,1] = (var + mean^2)/8
                nc.vector.tensor_scalar_mul(arin_sb[:, 0:1], mv[:, 0:1], 1.0 / 8)
                ex2 = small.tile([C, 1], F32, tag=f"ex2{name}")
                nc.vector.tensor_add(ex2[:], mv[:, 1:2], sq[:])
                nc.vector.tensor_scalar_mul(arin_sb[:, 1:2], ex2[:], 1.0 / 8)
                arin_d = dram.tile([C, 2], F32, tag=f"arin_d{name}")
                arout_d = dram.tile([C, 2], F32, tag=f"arout_d{name}")
                nc.gpsimd.dma_start(arin_d[:], arin_sb[:])
                nc.gpsimd.collective_compute(
                    "AllReduce", ALU.add, replica_groups=rg,
                    ins=[arin_d.opt()], outs=[arout_d.opt()],
                )
                gst = small.tile([C, 2], F32, tag=f"gst{name}")
                nc.gpsimd.dma_start(gst[:], arout_d[:])
                gm2 = small.tile([C, 1], F32, tag=f"gm2{name}")
                nc.vector.tensor_mul(gm2[:], gst[:, 0:1], gst[:, 0:1])
                gvar = small.tile([C, 1], F32, tag=f"gvar{name}")
                nc.vector.tensor_tensor(
                    out=gvar[:], in0=gst[:, 1:2], in1=gm2[:], op=ALU.subtract
                )
                std = small.tile([C, 1], F32, tag=f"std{name}")
                nc.scalar.activation(std[:], gvar[:], AF.Sqrt, bias=epst[:])
                rstd = small.tile([C, 1], F32, tag=f"rstd{name}")
                nc.vector.reciprocal(rstd[:], std[:])
                A = small.tile([C, 1], F32, tag=f"A{name}")
                nc.vector.tensor_mul(A[:], rstd[:], gamma[:])
                mA = small.tile([C, 1], F32, tag=f"mA{name}")
                nc.vector.tensor_mul(mA[:], gst[:, 0:1], A[:])
                B = small.tile([C, 1], F32, tag=f"B{name}")
                nc.vector.tensor_tensor(out=B[:], in0=beta[:], in1=mA[:], op=ALU.subtract)
                return A, B

            def masked_AB(A, B, msk, C, name):
                Am = small.tile([C, 1], F32, tag=f"Am{name}")
                Bm = small.tile([C, 1], F32, tag=f"Bm{name}")
                nc.vector.tensor_mul(Am[:], A[:], msk[:])
                nc.vector.tensor_mul(Bm[:], B[:], msk[:])
                return Am, Bm

            # ==================== Layer 1 ====================
            # conv1 1->64 via im2col (27 taps on K, 3 dg shifts accumulated).
            # Computes 5 d1 slices (1-slice redundant margin each side).
            T1 = hbig.tile([128, 5, 14, 14, 14], BF16, tag="h")
            nc.gpsimd.memset(T1[:], 0.0)
            st1 = stats.tile([64, 12, 6], F32, tag="st1")
            si = 0
            for d1p in [1, 2, 3, 0, 4]:  # owned slices first
                for d2g in range(N_D2G):
                    ps = psum.tile([64, 3, 12, 12], F32, tag="ps")
                    for dgi in range(3):
                        rhs = xc[:, d1p, 3 * d2g:3 * d2g + 3, :, dgi:dgi + 12]
                        nc.tensor.matmul(
                            ps[:], w1sb[:, dgi, :], rhs,
                            start=(dgi == 0), stop=(dgi == 2),
                        )
                    if d1p in (1, 2, 3):
                        nc.vector.bn_stats(st1[:, si, :], ff(ps[:]))
                        si += 1
                    nc.scalar.copy(
                        T1[0:64, d1p, 3 * d2g + 1:3 * d2g + 4, 1:13, 1:13], ps[:]
                    )
            mv1 = stats.tile([64, 2], F32, tag="mv1")
            nc.vector.bn_aggr(mv1[:], st1[:])
            A1, B1 = stats_to_AB(mv1, 64, g1sb, be1sb, eps64, RG_ALL, "1")
            A1L, B1L = masked_AB(A1, B1, ml64, 64, "1L")
            A1R, B1R = masked_AB(A1, B1, mr64, 64, "1R")
            for d1p, (a, b) in [
                (1, (A1, B1)), (2, (A1, B1)), (3, (A1, B1)),
                (0, (A1L, B1L)), (4, (A1R, B1R)),
            ]:
                win = T1[0:64, d1p, 1:13, 1:13, 1:13]
                nc.scalar.activation(win, win, AF.Relu, bias=b[:], scale=a[:])
                # dg-shifted copy for K=128 tap pairing: T1[64+c, s] = T1[c, s+1]
                nc.vector.tensor_copy(ff(T1[64:128, d1p]), ff(T1[0:64, d1p]))

            # ==================== Layer 2 ====================
            # conv2 64->128: 27 K=128 pair-matmuls (dg=-1,0) + 27 K=64 singles.
            h2 = hbig.tile([128, 5, 14, 14, 14], BF16, tag="h")
            nc.gpsimd.memset(h2[:], 0.0)
            st2 = stats.tile([128, 12, 6], F32, tag="st2")
            slab2 = slabs.tile([128, 2, 12, 12, 12], BF16, tag="slab")
            agin2 = dram.tile([2, 128, 12, 12, 12], BF16, tag="agin2")
            agout2 = dram.tile([4 * 2 * 128, 1728], BF16, tag="agout2")
            si = 0
            for d1o in [0, 2, 1]:
                for d2g in range(N_D2G):
                    psA = psum.tile([128, 3, 12, 12], F32, tag="ps")
                    psB = psum.tile([128, 3, 12, 12], F32, tag="ps")
                    for i in range(41):
                        for half in range(2):
                            t = 2 * i + half
                            if t > 80:
                                continue
                            dd, de, df, dg = (
                                t // 27, (t // 9) % 3, (t // 3) % 3, t % 3
                            )
                            lo = 64 * half
                            rhs = T1[lo:lo + 64, d1o + dd,
                                     3 * d2g + de:3 * d2g + de + 3,
                                     df:df + 12, dg:dg + 12]
                            pst = psA if half == 0 else psB
                            nc.tensor.matmul(
                                pst[:], w2asb[lo:lo + 64, t, :], rhs,
                                start=(i == 0), stop=(t >= 79),
                                tile_position=(lo, 0),
                            )
                    hrc = stats.tile([128, 3, 12, 12], F32, tag="hraw2")
                    nc.scalar.copy(hrc[:], psB[:])
                    nc.vector.tensor_tensor(
                        out=hrc[:], in0=hrc[:], in1=psA[:], op=ALU.add
                    )
                    nc.vector.bn_stats(st2[:, si, :], ff(hrc[:]))
                    si += 1
                    nc.scalar.copy(
                        h2[:, d1o + 1, 3 * d2g + 1:3 * d2g + 4, 1:13, 1:13], hrc[:]
                    )
                if d1o == 0:
                    nc.gpsimd.tensor_copy(slab2[:, 0], h2[:, 1, 1:13, 1:13, 1:13])
                    nc.gpsimd.dma_start(agin2[0], slab2[:, 0])
                elif d1o == 2:
                    nc.gpsimd.tensor_copy(slab2[:, 1], h2[:, 3, 1:13, 1:13, 1:13])
                    nc.gpsimd.dma_start(agin2[1], slab2[:, 1])
                    nc.gpsimd.collective_compute(
                        "AllGather", ALU.bypass, replica_groups=RG_N,
                        ins=[agin2.opt()], outs=[agout2.opt()],
                    )
            mv2 = stats.tile([128, 2], F32, tag="mv2")
            nc.vector.bn_aggr(mv2[:], st2[:])
            A2, B2 = stats_to_AB(mv2, 128, g2sb, be2sb, eps128, RG_ALL, "2")
            A2L, B2L = masked_AB(A2, B2, ml128, 128, "2L")
            A2R, B2R = masked_AB(A2, B2, mr128, 128, "2R")
            for d1p in [2, 1, 3]:
                win = h2[:, d1p, 1:13, 1:13, 1:13]
                nc.scalar.activation(win, win, AF.Relu, bias=B2[:], scale=A2[:])
            halo2 = slabs.tile([128, 2, 12, 12, 12], BF16, tag="halo")
            for s in range(2):
                nc.gpsimd.indirect_dma_start(
                    out=ff(halo2[:, s]),
                    out_offset=None,
                    in_=agout2[:],
                    in_offset=IndirectOffsetOnAxis(ap=hix128[:, s:s + 1], axis=0),
                )
            nc.scalar.activation(
                h2[:, 0, 1:13, 1:13, 1:13], halo2[:, 0], AF.Relu,
                bias=B2L[:], scale=A2L[:],
            )
            nc.scalar.activation(
                h2[:, 4, 1:13, 1:13, 1:13], halo2[:, 1], AF.Relu,
                bias=B2R[:], scale=A2R[:],
            )

            # ==================== Layer 3 ====================
            # conv3 128->64: K=128; M-packed x2 via col tile_position (0,0)/(0,64)
            w3sb = wpool.tile([128, 81, 64], BF16, tag="wa")
            nc.sync.dma_start(w3sb[:], w3t[:])
            h3 = hbig.tile([128, 5, 14, 14, 14], BF16, tag="h")
            nc.gpsimd.memset(h3[:], 0.0)
            hraw3 = stats.tile([64, 3, 4, 3, 12, 12], F32, tag="hraw3")  # [d1o][d2g]
            st3 = stats.tile([64, 12, 6], F32, tag="st3")
            slab3 = slabs.tile([64, 2, 12, 12, 12], BF16, tag="slab")
            agin3 = dram.tile([2, 64, 12, 12, 12], BF16, tag="agin3")
            agout3 = dram.tile([4 * 2 * 64, 1728], BF16, tag="agout3")
            si = 0
            for d1o in [0, 2, 1]:
                for d2g in range(N_D2G):
                    ps = psum.tile([128, 3, 12, 12], F32, tag="ps")
                    for i in range(41):
                        for half in range(2):
                            t = 2 * i + half
                            if t > 80:
                                continue
                            dd, de, df, dg = (
                                t // 27, (t // 9) % 3, (t // 3) % 3, t % 3
                            )
                            rhs = h2[:, d1o + dd, 3 * d2g + de:3 * d2g + de + 3,
                                     df:df + 12, dg:dg + 12]
                            nc.tensor.matmul(
                                ps[64 * half:64 * half + 64, :],
                                w3sb[:, t, :], rhs,
                                start=(i == 0), stop=(t >= 79),
                                tile_position=(0, 64 * half),
                            )
                    nc.scalar.copy(hraw3[:, d1o, d2g], ps[64:128, :])
                    nc.vector.tensor_tensor(
                        out=hraw3[:, d1o, d2g], in0=hraw3[:, d1o, d2g],
                        in1=ps[0:64, :], op=ALU.add,
                    )
                    nc.vector.bn_stats(st3[:, si, :], ff(hraw3[:, d1o, d2g]))
                    si += 1
                if d1o == 0:
                    nc.gpsimd.tensor_copy(ff(slab3[:, 0]), ff(hraw3[:, 0]))
                    nc.gpsimd.dma_start(agin3[0], slab3[:, 0])
                elif d1o == 2:
                    nc.gpsimd.tensor_copy(ff(slab3[:, 1]), ff(hraw3[:, 2]))
                    nc.gpsimd.dma_start(agin3[1], slab3[:, 1])
                    nc.gpsimd.collective_compute(
                        "AllGather", ALU.bypass, replica_groups=RG_N,
                        ins=[agin3.opt()], outs=[agout3.opt()],
                    )
            mv3 = stats.tile([64, 2], F32, tag="mv3")
            nc.vector.bn_aggr(mv3[:], st3[:])
            A3, B3 = stats_to_AB(mv3, 64, g3sb, be3sb, eps64, RG_ALL, "3")
            A3L, B3L = masked_AB(A3, B3, ml64, 64, "3L")
            A3R, B3R = masked_AB(A3, B3, mr64, 64, "3R")
            for d1o in [1, 0, 2]:
                for d2g in range(N_D2G):
                    nc.scalar.activation(
                        h3[0:64, d1o + 1, 3 * d2g + 1:3 * d2g + 4, 1:13, 1:13],
                        hraw3[:, d1o, d2g], AF.Relu, bias=B3[:], scale=A3[:],
                    )
                nc.vector.tensor_copy(
                    ff(h3[64:128, d1o + 1]), ff(h3[0:64, d1o + 1])
                )
            halo3 = slabs.tile([64, 2, 12, 12, 12], BF16, tag="halo")
            for s in range(2):
                nc.gpsimd.indirect_dma_start(
                    out=ff(halo3[:, s]),
                    out_offset=None,
                    in_=agout3[:],
                    in_offset=IndirectOffsetOnAxis(ap=hix64[:, s:s + 1], axis=0),
                )
            nc.scalar.activation(
                h3[0:64, 0, 1:13, 1:13, 1:13], halo3[:, 0], AF.Relu,
                bias=B3L[:], scale=A3L[:],
            )
            nc.scalar.activation(
                h3[0:64, 4, 1:13, 1:13, 1:13], halo3[:, 1], AF.Relu,
                bias=B3R[:], scale=A3R[:],
            )
            nc.vector.tensor_copy(ff(h3[64:128, 0]), ff(h3[0:64, 0]))
            nc.vector.tensor_copy(ff(h3[64:128, 4]), ff(h3[0:64, 4]))

            # ==================== Layer 4 ====================
            # conv4 64->1 + sigmoid: M=1, col-packed x4 at partitions 0/32/64/96
            w4sb = wpool.tile([128, 81], BF16, tag="wb")
            nc.sync.dma_start(w4sb[:], w4t[:])
            y4 = stats.tile([1, 3, 4, 3, 12, 12], BF16, tag="hraw3")
            for d1o in range(3):
                for d2g in range(N_D2G):
                    psA = psum.tile([128, 3, 12, 12], F32, tag="ps")
                    psB = psum.tile([128, 3, 12, 12], F32, tag="ps")
                    started = [False] * 8
                    for t in range(81):
                        j = t % 8
                        half, col = j // 4, j % 4
                        lo = 64 * half
                        dd, de, df, dg = t // 27, (t // 9) % 3, (t // 3) % 3, t % 3
                        rhs = h3[lo:lo + 64, d1o + dd,
                                 3 * d2g + de:3 * d2g + de + 3,
                                 df:df + 12, dg:dg + 12]
                        pst = psA if half == 0 else psB
                        nc.tensor.matmul(
                            pst[32 * col:32 * col + 1, :],
                            w4sb[lo:lo + 64, t:t + 1], rhs,
                            start=(not started[j]), stop=(t >= 73),
                            tile_position=(lo, 32 * col),
                        )
                        started[j] = True
                    u1 = small.tile([1, 3, 12, 12], F32, tag="u1")
                    nc.scalar.copy(u1[:], psA[0:1, :])
                    for pst, pj in ((psA, 32), (psA, 64), (psA, 96),
                                    (psB, 0), (psB, 32), (psB, 64), (psB, 96)):
                        nc.vector.tensor_tensor(
                            out=u1[:], in0=u1[:], in1=pst[pj:pj + 1, :], op=ALU.add
                        )
                    nc.scalar.activation(
                        y4[:, d1o, d2g], u1[:], AF.Sigmoid, bias=b4sb[:]
                    )
            # y4 [1, d1o, d2g, 3, 12, 12] -> yout [1, 3, 4, 3, 12, 12]
            nc.sync.dma_start(yout.ap(), y4[:])

    nc.compile()
    return nc


_CACHE = {}
_BF = ml_dtypes.bfloat16


def _prep_static(w1, w2, w3, w4, g1, be1, g2, be2, g3, be3, b4):
    """Per-input-name concat arrays [8*dim0, ...] for everything except xps."""
    w1t = np.ascontiguousarray(
        np.transpose(np.asarray(w1, np.float32)[:, 0], (1, 2, 3, 4, 0))
    ).reshape(27, 3, 64)
    wt2 = np.transpose(np.asarray(w2, np.float32), (1, 2, 3, 4, 5, 0)).reshape(
        64, 81, 128
    )
    w2a = np.ascontiguousarray(np.concatenate([wt2, wt2], axis=0))  # [128,81,128]
    w3t = np.ascontiguousarray(
        np.transpose(np.asarray(w3, np.float32), (1, 2, 3, 4, 5, 0)).reshape(
            128, 81, 64
        )
    )
    w4t = np.asarray(w4, np.float32)[0].reshape(64, 81)
    w4t = np.ascontiguousarray(np.concatenate([w4t, w4t], axis=0))

    def rep(a):
        return np.ascontiguousarray(
            np.broadcast_to(a[None], (N_CORES, *a.shape)).reshape(
                N_CORES * a.shape[0], *a.shape[1:]
            )
        )

    out = {
        "w1t": rep(w1t.astype(_BF)), "w2a": rep(w2a.astype(_BF)),
        "w3t": rep(w3t.astype(_BF)), "w4t": rep(w4t.astype(_BF)),
        "g1": rep(np.asarray(g1, np.float32).reshape(64, 1)),
        "be1": rep(np.asarray(be1, np.float32).reshape(64, 1)),
        "g2": rep(np.asarray(g2, np.float32).reshape(128, 1)),
        "be2": rep(np.asarray(be2, np.float32).reshape(128, 1)),
        "g3": rep(np.asarray(g3, np.float32).reshape(64, 1)),
        "be3": rep(np.asarray(be3, np.float32).reshape(64, 1)),
        "b4": rep(np.asarray(b4, np.float32).reshape(1, 1)),
    }
    # per-core constants: masks + halo gather indices
    mlv = np.zeros((N_CORES, 1, 1), np.float32)
    mrv = np.zeros((N_CORES, 1, 1), np.float32)
    hidx128 = np.zeros((N_CORES, 128, 2), np.int32)
    hidx64 = np.zeros((N_CORES, 64, 2), np.int32)
    for c in range(N_CORES):
        q = c % 4
        mlv[c] = 0.0 if q == 0 else 1.0
        mrv[c] = 0.0 if q == 3 else 1.0
        ql, qr = (q - 1) % 4, (q + 1) % 4
        hidx128[c, :, 0] = (2 * ql + 1) * 128 + np.arange(128)
        hidx128[c, :, 1] = (2 * qr + 0) * 128 + np.arange(128)
        hidx64[c, :, 0] = (2 * ql + 1) * 64 + np.arange(64)
        hidx64[c, :, 1] = (2 * qr + 0) * 64 + np.arange(64)
    out["ml"] = mlv.reshape(N_CORES * 1, 1)
    out["mr"] = mrv.reshape(N_CORES * 1, 1)
    out["hidx128"] = hidx128.reshape(N_CORES * 128, 2)
    out["hidx64"] = hidx64.reshape(N_CORES * 64, 2)
    return out


def _get_state():
    if "st" in _CACHE:
        return _CACHE["st"]
    import jax
    from jax.sharding import Mesh, PartitionSpec, NamedSharding
    from jax.experimental.shard_map import shard_map
    from concourse import bass2jax

    nc = _build_module()
    bass2jax.install_neuronx_cc_hook()
    partition_name = nc.partition_id_tensor.name if nc.partition_id_tensor else None
    in_names, out_names, out_avals, zero_outs = [], [], [], []
    for alloc in nc.m.functions[0].allocations:
        if not isinstance(alloc, mybir.MemoryLocationSet):
            continue
        name = alloc.memorylocations[0].name
        if alloc.kind == "ExternalInput":
            if name != partition_name:
                in_names.append(name)
        elif alloc.kind == "ExternalOutput":
            shape = tuple(alloc.tensor_shape)
            dtype = mybir.dt.np(alloc.dtype)
            out_names.append(name)
            out_avals.append(jax.core.ShapedArray(shape, dtype))
            zero_outs.append(np.zeros(shape, dtype))
    n_params, n_outs = len(in_names), len(out_avals)
    bind_names = tuple(
        in_names + out_names + ([partition_name] if partition_name else []))

    def _body(*args):
        operands = list(args)
        if partition_name is not None:
            operands.append(bass2jax.partition_id_tensor())
        outs = bass2jax._bass_exec_p.bind(
            *operands, out_avals=tuple(out_avals), in_names=bind_names,
            out_names=tuple(out_names), lowering_input_output_aliases=(),
            sim_require_finite=True, sim_require_nnan=True, nc=nc)
        return tuple(outs)

    mesh = Mesh(np.asarray(jax.devices()[:N_CORES]), ("core",))
    sharding = NamedSharding(mesh, PartitionSpec("core"))
    in_specs = (PartitionSpec("core"),) * (n_params + n_outs)
    out_specs = (PartitionSpec("core"),) * n_outs

    # example args (zeros) — compile is weight-independent
    example_in = []
    for alloc in nc.m.functions[0].allocations:
        if not isinstance(alloc, mybir.MemoryLocationSet):
            continue
        if alloc.kind == "ExternalInput":
            name = alloc.memorylocations[0].name
            if name == partition_name:
                continue
            shape = tuple(alloc.tensor_shape)
            example_in.append(
                np.zeros((N_CORES * shape[0], *shape[1:]), mybir.dt.np(alloc.dtype))
            )
    concat_zeros = [
        np.zeros((N_CORES * z.shape[0], *z.shape[1:]), z.dtype) for z in zero_outs
    ]
    # No donation: yout is fully written by the kernel, so the pre-zeroed
    # output operand's content is never observed and can stay device-resident.
    try:
        compiled = bass2jax.fast_dispatch_compile(
            lambda: jax.jit(
                shard_map(_body, mesh=mesh, in_specs=in_specs,
                          out_specs=out_specs, check_rep=False),
                keep_unused=True,
            ).lower(*example_in, *concat_zeros).compile()
        )
        resident_zeros = [jax.device_put(z, sharding) for z in concat_zeros]
        jax.block_until_ready(resident_zeros)
    except Exception:
        compiled, resident_zeros = None, None
    _CACHE["concat_zeros_np"] = concat_zeros
    st = {
        "jax": jax,
        "nc": nc,
        "in_names": in_names,
        "compiled": compiled,
        "sharding": sharding,
        "resident_zeros": resident_zeros,
        "static_ids": None,
        "static_np": None,
        "static_dev": None,
        "static_concat": None,
    }
    _CACHE["st"] = st
    return st


def _ensure_static(st, statics):
    """statics: tuple of the 11 weight/param arrays in fixed order."""
    ids = tuple(id(a) for a in statics)
    if st["static_ids"] == ids:
        return
    if st["static_np"] is not None and all(
        np.array_equal(a, b) for a, b in zip(st["static_np"], statics)
    ):
        st["static_ids"] = ids
        return
    concat = _prep_static(*statics)
    st["static_concat"] = concat
    if st["compiled"] is not None:
        dev = {k: st["jax"].device_put(v, st["sharding"]) for k, v in concat.items()}
        st["jax"].block_until_ready(list(dev.values()))
        st["static_dev"] = dev
    st["static_np"] = tuple(np.asarray(a) for a in statics)
    st["static_ids"] = ids


def _run_fallback(st, xps):
    """Per-call dispatch through run_bass_kernel_spmd (AOT compile failed)."""
    from concourse.bass_utils import run_bass_kernel_spmd

    concat = st["static_concat"]
    in_maps = []
    for c in range(N_CORES):
        m = {}
        for nm, arr in concat.items():
            d0 = arr.shape[0] // N_CORES
            m[nm] = arr[c * d0:(c + 1) * d0]
        m["xps"] = xps[c:c + 1]
        in_maps.append(m)
    res = run_bass_kernel_spmd(st["nc"], in_maps, core_ids=list(range(N_CORES)))
    return np.stack([r["yout"][0] for r in res.results])


def kernel(x, w1, b1, g1, be1, w2, b2, g2, be2, w3, b3, g3, be3, w4, b4):
    # b1/b2/b3 cancel inside training-mode BN; b4 is applied before sigmoid.
    statics = (w1, w2, w3, w4, g1, be1, g2, be2, g3, be3, b4)

    # compact padded x shards: d1 pad 2 (margin windows reach d1 in [-2,13]),
    # e/f/g pad 1. Core (n,q) gets padded-d1 slices [3q, 3q+7).
    x = np.asarray(x, np.float32)
    xp = np.pad(x[:, 0], ((0, 0), (2, 2), (1, 1), (1, 1), (1, 1)))
    xps = np.empty((N_CORES, 7, 14, 14, 14), np.float32)
    for c in range(N_CORES):
        n, q = c // 4, c % 4
        xps[c] = xp[n, 3 * q:3 * q + 7]
    xps = xps.astype(_BF)

    y = None
    for attempt in range(4):
        try:
            st = _get_state()
            _ensure_static(st, statics)
            if st["compiled"] is None:
                y = _run_fallback(st, xps)
            else:
                args = [xps if nm == "xps" else st["static_dev"][nm]
                        for nm in st["in_names"]]
                outs = st["compiled"](*args, *st["resident_zeros"])
                y = np.asarray(outs[0])
            break
        except Exception:
            if attempt == 0:
                continue  # transient: immediate retry
            if attempt >= 3:
                raise
            # axon worker may have crashed; it self-restarts in ~60s and all
            # device state (buffers, executable) dies with it. Full rebuild.
            import time as _time

            _time.sleep(75)
            try:
                from jax._src import xla_bridge as _xb

                _xb._clear_backends()
            except Exception:
                pass
            _CACHE.clear()

    y = y.astype(np.float32).reshape(N_CORES, 3, 4, 3, 12, 12)
    out = np.empty((2, 1, 12, 12, 12, 12), np.float32)
    for c in range(N_CORES):
        n, q = c // 4, c % 4
        out[n, 0, 3 * q:3 * q + 3] = y[c].reshape(3, 12, 12, 12)
    return out


# revision 19
# speedup vs baseline: 2.5294x; 2.5294x over previous
"""Trainium2 Bass kernel for ComplexConv4dNet (4-layer 4D CNN + training-mode BN).

Sharding: 8 cores = N(2) x D1-quarters(4, 3 slices each).
Per core all activations live in SBUF, padded layout [C, 5, 14, 14, 14]
(d1: 3 owned + 2 halo; d2/d3/d4: 12 + 1 zero-pad each side).
Each conv tap = accumulating PE matmul over a shifted window view.
BN stats: bn_stats on psum chunks -> AllReduce of (mean/8, E[x^2]/8).
Halos: L1 computes a 1-slice margin redundantly (no exchange); h2/h3 halos go
over a bf16 slab AllGather (groups of 4 same-n cores) + indirect-DMA gather,
with edge cores masking their out-of-domain halo slices to zero.

Host/dispatch layer: one AOT-compiled shard_map executable cached across
calls; weights + per-core constants stay device-resident; the only per-call
upload is the compact padded x shard (im2col happens on device via 27 DMAs);
output travels back as bf16.
"""

import ml_dtypes
import numpy as np

import concourse.bass as bass
import concourse.mybir as mybir
import concourse.tile as tile
from concourse import bacc
from concourse.bass import IndirectOffsetOnAxis

N_CORES = 8
D = 12
EPS = 1e-5
F32 = mybir.dt.float32
BF16 = mybir.dt.bfloat16
I32 = mybir.dt.int32
AF = mybir.ActivationFunctionType
ALU = mybir.AluOpType

# chunking: free chunk = (d1 slice, group of 3 d2 rows) -> [3,12,12] = 432
N_D2G = 4


def ff(ap):
    """Flatten the free (non-partition) dims of an AP."""
    n = len(ap.shape) - 1
    names = " ".join(f"d{i}" for i in range(n))
    return ap.rearrange(f"p {names} -> p ({names})")


def _build_module():
    nc = bacc.Bacc(None, target_bir_lowering=False)

    # ---- kernel I/O ----
    # xps: padded x shard, d1 slices [3q .. 3q+7) of the (2,2)-padded d1 axis,
    # full 14^3 padded e/f/g. The 27-way im2col expansion happens on device.
    xps = nc.dram_tensor("xps", [1, 7, 14, 14, 14], BF16, kind="ExternalInput")
    w1 = nc.dram_tensor("w1t", [27, 3, 64], BF16, kind="ExternalInput")
    w2a = nc.dram_tensor("w2a", [128, 81, 128], BF16, kind="ExternalInput")
    w3t = nc.dram_tensor("w3t", [128, 81, 64], BF16, kind="ExternalInput")
    w4t = nc.dram_tensor("w4t", [128, 54], BF16, kind="ExternalInput")
    g1 = nc.dram_tensor("g1", [64, 1], F32, kind="ExternalInput")
    be1 = nc.dram_tensor("be1", [64, 1], F32, kind="ExternalInput")
    g2 = nc.dram_tensor("g2", [128, 1], F32, kind="ExternalInput")
    be2 = nc.dram_tensor("be2", [128, 1], F32, kind="ExternalInput")
    g3 = nc.dram_tensor("g3", [64, 1], F32, kind="ExternalInput")
    be3 = nc.dram_tensor("be3", [64, 1], F32, kind="ExternalInput")
    b4 = nc.dram_tensor("b4", [1, 1], F32, kind="ExternalInput")
    ml = nc.dram_tensor("ml", [1, 1], F32, kind="ExternalInput")  # 0 if q==0
    mr = nc.dram_tensor("mr", [1, 1], F32, kind="ExternalInput")  # 0 if q==3
    hidx128 = nc.dram_tensor("hidx128", [128, 2], I32, kind="ExternalInput")
    hidx64 = nc.dram_tensor("hidx64", [64, 2], I32, kind="ExternalInput")
    yout = nc.dram_tensor("yout", [1, 3, 4, 3, 12, 12], BF16, kind="ExternalOutput")

    RG_ALL = [list(range(N_CORES))]
    RG_N = [[0, 1, 2, 3], [4, 5, 6, 7]]

    with tile.TileContext(nc) as tc:
        with (
            tc.tile_pool(name="consts", bufs=1) as consts,
            tc.tile_pool(name="hbig", bufs=2) as hbig,
            tc.tile_pool(name="wpool", bufs=1) as wpool,
            tc.tile_pool(name="psum", bufs=6, space="PSUM") as psum,
            tc.tile_pool(name="stats", bufs=1) as stats,
            tc.tile_pool(name="slabs", bufs=1) as slabs,
            tc.tile_pool(name="small", bufs=2) as small,
            tc.tile_pool(name="dram", bufs=1, space="DRAM") as dram,
        ):
            # ---- load constants + device-side im2col of the x shard ----
            # xc[t3] = xps[0, dd:dd+5, de:de+12, df:df+12, :] for t3=(dd,de,df)
            xc = hbig.tile([27, 5, 12, 12, 14], BF16, tag="h")
            im2col_engines = [nc.sync, nc.scalar, nc.gpsimd]
            for t3 in range(27):
                dd, de, df = t3 // 9, (t3 // 3) % 3, t3 % 3
                im2col_engines[t3 % 3].dma_start(
                    xc[t3:t3 + 1],
                    xps[0:1, dd:dd + 5, de:de + 12, df:df + 12, :],
                )
            w1sb = consts.tile([27, 3, 64], BF16)
            nc.sync.dma_start(w1sb[:], w1[:])
            w2asb = wpool.tile([128, 81, 128], BF16, tag="wa")
            nc.sync.dma_start(w2asb[:], w2a[:])

            def bc_load(handle, p):
                t = consts.tile([p, 1], F32, tag=f"bc_{handle.name}_{p}")
                nc.sync.dma_start(t[:], handle.ap().to_broadcast([p, 1]))
                return t

            g1sb, be1sb = bc_load(g1, 64), bc_load(be1, 64)
            g2sb, be2sb = bc_load(g2, 128), bc_load(be2, 128)
            g3sb, be3sb = bc_load(g3, 64), bc_load(be3, 64)
            b4sb = bc_load(b4, 1)
            ml64, mr64 = bc_load(ml, 64), bc_load(mr, 64)
            ml128, mr128 = bc_load(ml, 128), bc_load(mr, 128)
            hix128 = consts.tile([128, 2], I32)
            nc.sync.dma_start(hix128[:], hidx128[:])
            hix64 = consts.tile([64, 2], I32)
            nc.sync.dma_start(hix64[:], hidx64[:])

            eps64 = consts.tile([64, 1], F32)
            nc.vector.memset(eps64[:], EPS)
            eps128 = consts.tile([128, 1], F32)
            nc.vector.memset(eps128[:], EPS)

            # -------- helpers --------
            def stats_to_AB(mv, C, gamma, beta, epst, rg, name):
                """mv [C,2] = (mean, var) over the local 5184 owned voxels.
                AllReduce (mean/8, E[x^2]/8) -> global (A, B) with
                A = gamma * rsqrt(var + eps), B = beta - mean * A."""
                sq = small.tile([C, 1], F32, tag=f"sq{name}")
                nc.vector.tensor_mul(sq[:], mv[:, 0:1], mv[:, 0:1])
                arin_sb = small.tile([C, 2], F32, tag=f"arin{name}")
                # arin[:,0] = mean/8 ; arin[:,1] = (var + mean^2)/8
                nc.vector.tensor_scalar_mul(arin_sb[:, 0:1], mv[:, 0:1], 1.0 / 8)
                ex2 = small.tile([C, 1], F32, tag=f"ex2{name}")
                nc.vector.tensor_add(ex2[:], mv[:, 1:2], sq[:])
                nc.vector.tensor_scalar_mul(arin_sb[:, 1:2], ex2[:], 1.0 / 8)
                arin_d = dram.tile([C, 2], F32, tag=f"arin_d{name}")
                arout_d = dram.tile([C, 2], F32, tag=f"arout_d{name}")
                nc.gpsimd.dma_start(arin_d[:], arin_sb[:])
                nc.gpsimd.collective_compute(
                    "AllReduce", ALU.add, replica_groups=rg,
                    ins=[arin_d.opt()], outs=[arout_d.opt()],
                )
                gst = small.tile([C, 2], F32, tag=f"gst{name}")
                nc.gpsimd.dma_start(gst[:], arout_d[:])
                gm2 = small.tile([C, 1], F32, tag=f"gm2{name}")
                nc.vector.tensor_mul(gm2[:], gst[:, 0:1], gst[:, 0:1])
                gvar = small.tile([C, 1], F32, tag=f"gvar{name}")
                nc.vector.tensor_tensor(
                    out=gvar[:], in0=gst[:, 1:2], in1=gm2[:], op=ALU.subtract
                )
                std = small.tile([C, 1], F32, tag=f"std{name}")
                nc.scalar.activation(std[:], gvar[:], AF.Sqrt, bias=epst[:])
                rstd = small.tile([C, 1], F32, tag=f"rstd{name}")
                nc.vector.reciprocal(rstd[:], std[:])
                A = small.tile([C, 1], F32, tag=f"A{name}")
                nc.vector.tensor_mul(A[:], rstd[:], gamma[:])
                mA = small.tile([C, 1], F32, tag=f"mA{name}")
                nc.vector.tensor_mul(mA[:], gst[:, 0:1], A[:])
                B = small.tile([C, 1], F32, tag=f"B{name}")
                nc.vector.tensor_tensor(out=B[:], in0=beta[:], in1=mA[:], op=ALU.subtract)
                return A, B

            def masked_AB(A, B, msk, C, name):
                Am = small.tile([C, 1], F32, tag=f"Am{name}")
                Bm = small.tile([C, 1], F32, tag=f"Bm{name}")
                nc.vector.tensor_mul(Am[:], A[:], msk[:])
                nc.vector.tensor_mul(Bm[:], B[:], msk[:])
                return Am, Bm

            # ==================== Layer 1 ====================
            # conv1 1->64 via im2col (27 taps on K, 3 dg shifts accumulated).
            # Computes 5 d1 slices (1-slice redundant margin each side).
            T1 = hbig.tile([128, 5, 14, 14, 14], BF16, tag="h")
            nc.gpsimd.memset(T1[:], 0.0)
            st1 = stats.tile([64, 12, 6], F32, tag="st1")
            si = 0
            for d1p in [1, 2, 3, 0, 4]:  # owned slices first
                for d2g in range(N_D2G):
                    ps = psum.tile([64, 3, 12, 12], F32, tag="ps")
                    for dgi in range(3):
                        rhs = xc[:, d1p, 3 * d2g:3 * d2g + 3, :, dgi:dgi + 12]
                        nc.tensor.matmul(
                            ps[:], w1sb[:, dgi, :], rhs,
                            start=(dgi == 0), stop=(dgi == 2),
                        )
                    if d1p in (1, 2, 3):
                        nc.vector.bn_stats(st1[:, si, :], ff(ps[:]))
                        si += 1
                    nc.scalar.copy(
                        T1[0:64, d1p, 3 * d2g + 1:3 * d2g + 4, 1:13, 1:13], ps[:]
                    )
            mv1 = stats.tile([64, 2], F32, tag="mv1")
            nc.vector.bn_aggr(mv1[:], st1[:])
            A1, B1 = stats_to_AB(mv1, 64, g1sb, be1sb, eps64, RG_ALL, "1")
            A1L, B1L = masked_AB(A1, B1, ml64, 64, "1L")
            A1R, B1R = masked_AB(A1, B1, mr64, 64, "1R")
            for d1p, (a, b) in [
                (1, (A1, B1)), (2, (A1, B1)), (3, (A1, B1)),
                (0, (A1L, B1L)), (4, (A1R, B1R)),
            ]:
                win = T1[0:64, d1p, 1:13, 1:13, 1:13]
                nc.scalar.activation(win, win, AF.Relu, bias=b[:], scale=a[:])
                # dg-shifted copy for K=128 tap pairing: T1[64+c, s] = T1[c, s+1]
                nc.vector.tensor_copy(ff(T1[64:128, d1p]), ff(T1[0:64, d1p]))

            # ==================== Layer 2 ====================
            # conv2 64->128: 27 K=128 pair-matmuls (dg=-1,0) + 27 K=64 singles.
            h2 = hbig.tile([128, 5, 14, 14, 14], BF16, tag="h")
            nc.gpsimd.memset(h2[:], 0.0)
            st2 = stats.tile([128, 12, 6], F32, tag="st2")
            slab2 = slabs.tile([128, 2, 12, 12, 12], BF16, tag="slab")
            agin2 = dram.tile([2, 128, 12, 12, 12], BF16, tag="agin2")
            agout2 = dram.tile([4 * 2 * 128, 1728], BF16, tag="agout2")
            si = 0
            for d1o in [0, 2, 1]:
                for d2g in range(N_D2G):
                    psA = psum.tile([128, 3, 12, 12], F32, tag="ps")
                    psB = psum.tile([128, 3, 12, 12], F32, tag="ps")
                    for i in range(41):
                        for half in range(2):
                            t = 2 * i + half
                            if t > 80:
                                continue
                            dd, de, df, dg = (
                                t // 27, (t // 9) % 3, (t // 3) % 3, t % 3
                            )
                            lo = 64 * half
                            rhs = T1[lo:lo + 64, d1o + dd,
                                     3 * d2g + de:3 * d2g + de + 3,
                                     df:df + 12, dg:dg + 12]
                            pst = psA if half == 0 else psB
                            nc.tensor.matmul(
                                pst[:], w2asb[lo:lo + 64, t, :], rhs,
                                start=(i == 0), stop=(t >= 79),
                                tile_position=(lo, 0),
                            )
                    hrc = stats.tile([128, 3, 12, 12], F32, tag="hraw2")
                    nc.scalar.copy(hrc[:], psB[:])
                    nc.vector.tensor_tensor(
                        out=hrc[:], in0=hrc[:], in1=psA[:], op=ALU.add
                    )
                    nc.vector.bn_stats(st2[:, si, :], ff(hrc[:]))
                    si += 1
                    nc.scalar.copy(
                        h2[:, d1o + 1, 3 * d2g + 1:3 * d2g + 4, 1:13, 1:13], hrc[:]
                    )
                if d1o == 0:
                    nc.gpsimd.tensor_copy(slab2[:, 0], h2[:, 1, 1:13, 1:13, 1:13])
                    nc.gpsimd.dma_start(agin2[0], slab2[:, 0])
                elif d1o == 2:
                    nc.gpsimd.tensor_copy(slab2[:, 1], h2[:, 3, 1:13, 1:13, 1:13])
                    nc.gpsimd.dma_start(agin2[1], slab2[:, 1])
                    nc.gpsimd.collective_compute(
                        "AllGather", ALU.bypass, replica_groups=RG_N,
                        ins=[agin2.opt()], outs=[agout2.opt()],
                    )
            mv2 = stats.tile([128, 2], F32, tag="mv2")
            nc.vector.bn_aggr(mv2[:], st2[:])
            A2, B2 = stats_to_AB(mv2, 128, g2sb, be2sb, eps128, RG_ALL, "2")
            A2L, B2L = masked_AB(A2, B2, ml128, 128, "2L")
            A2R, B2R = masked_AB(A2, B2, mr128, 128, "2R")
            for d1p in [2, 1, 3]:
                win = h2[:, d1p, 1:13, 1:13, 1:13]
                nc.scalar.activation(win, win, AF.Relu, bias=B2[:], scale=A2[:])
            halo2 = slabs.tile([128, 2, 12, 12, 12], BF16, tag="halo")
            for s in range(2):
                nc.gpsimd.indirect_dma_start(
                    out=ff(halo2[:, s]),
                    out_offset=None,
                    in_=agout2[:],
                    in_offset=IndirectOffsetOnAxis(ap=hix128[:, s:s + 1], axis=0),
                )
            nc.scalar.activation(
                h2[:, 0, 1:13, 1:13, 1:13], halo2[:, 0], AF.Relu,
                bias=B2L[:], scale=A2L[:],
            )
            nc.scalar.activation(
                h2[:, 4, 1:13, 1:13, 1:13], halo2[:, 1], AF.Relu,
                bias=B2R[:], scale=A2R[:],
            )

            # ==================== Layer 3 ====================
            # conv3 128->64: K=128; M-packed x2 via col tile_position (0,0)/(0,64)
            w3sb = wpool.tile([128, 81, 64], BF16, tag="wa")
            nc.sync.dma_start(w3sb[:], w3t[:])
            h3 = hbig.tile([128, 5, 14, 14, 14], BF16, tag="h")
            nc.gpsimd.memset(h3[:], 0.0)
            hraw3 = stats.tile([64, 3, 4, 3, 12, 12], F32, tag="hraw3")  # [d1o][d2g]
            st3 = stats.tile([64, 12, 6], F32, tag="st3")
            slab3 = slabs.tile([64, 2, 12, 12, 12], BF16, tag="slab")
            agin3 = dram.tile([2, 64, 12, 12, 12], BF16, tag="agin3")
            agout3 = dram.tile([4 * 2 * 64, 1728], BF16, tag="agout3")
            si = 0
            for d1o in [0, 2, 1]:
                for d2g in range(N_D2G):
                    ps = psum.tile([128, 3, 12, 12], F32, tag="ps")
                    for i in range(41):
                        for half in range(2):
                            t = 2 * i + half
                            if t > 80:
                                continue
                            dd, de, df, dg = (
                                t // 27, (t // 9) % 3, (t // 3) % 3, t % 3
                            )
                            rhs = h2[:, d1o + dd, 3 * d2g + de:3 * d2g + de + 3,
                                     df:df + 12, dg:dg + 12]
                            nc.tensor.matmul(
                                ps[64 * half:64 * half + 64, :],
                                w3sb[:, t, :], rhs,
                                start=(i == 0), stop=(t >= 79),
                                tile_position=(0, 64 * half),
                            )
                    nc.scalar.copy(hraw3[:, d1o, d2g], ps[64:128, :])
                    nc.vector.tensor_tensor(
                        out=hraw3[:, d1o, d2g], in0=hraw3[:, d1o, d2g],
                        in1=ps[0:64, :], op=ALU.add,
                    )
                    nc.vector.bn_stats(st3[:, si, :], ff(hraw3[:, d1o, d2g]))
                    si += 1
                if d1o == 0:
                    nc.gpsimd.tensor_copy(ff(slab3[:, 0]), ff(hraw3[:, 0]))
                    nc.gpsimd.dma_start(agin3[0], slab3[:, 0])
                elif d1o == 2:
                    nc.gpsimd.tensor_copy(ff(slab3[:, 1]), ff(hraw3[:, 2]))
                    nc.gpsimd.dma_start(agin3[1], slab3[:, 1])
                    nc.gpsimd.collective_compute(
                        "AllGather", ALU.bypass, replica_groups=RG_N,
                        ins=[agin3.opt()], outs=[agout3.opt()],
                    )
            mv3 = stats.tile([64, 2], F32, tag="mv3")
            nc.vector.bn_aggr(mv3[:], st3[:])
            A3, B3 = stats_to_AB(mv3, 64, g3sb, be3sb, eps64, RG_ALL, "3")
            A3L, B3L = masked_AB(A3, B3, ml64, 64, "3L")
            A3R, B3R = masked_AB(A3, B3, mr64, 64, "3R")
            for d1o in [1, 0, 2]:
                for d2g in range(N_D2G):
                    nc.scalar.activation(
                        h3[0:64, d1o + 1, 3 * d2g + 1:3 * d2g + 4, 1:13, 1:13],
                        hraw3[:, d1o, d2g], AF.Relu, bias=B3[:], scale=A3[:],
                    )
                # g-shifted copy for L4 tap-pair K-folding:
                # h3[64+c, s, b, f, g] = h3[c, s, b, f, g+1]; col 13 stays 0.
                nc.vector.tensor_copy(
                    h3[64:128, d1o + 1, :, :, 0:13], h3[0:64, d1o + 1, :, :, 1:14]
                )
            halo3 = slabs.tile([64, 2, 12, 12, 12], BF16, tag="halo")
            for s in range(2):
                nc.gpsimd.indirect_dma_start(
                    out=ff(halo3[:, s]),
                    out_offset=None,
                    in_=agout3[:],
                    in_offset=IndirectOffsetOnAxis(ap=hix64[:, s:s + 1], axis=0),
                )
            nc.scalar.activation(
                h3[0:64, 0, 1:13, 1:13, 1:13], halo3[:, 0], AF.Relu,
                bias=B3L[:], scale=A3L[:],
            )
            nc.scalar.activation(
                h3[0:64, 4, 1:13, 1:13, 1:13], halo3[:, 1], AF.Relu,
                bias=B3R[:], scale=A3R[:],
            )
            nc.vector.tensor_copy(h3[64:128, 0, :, :, 0:13], h3[0:64, 0, :, :, 1:14])
            nc.vector.tensor_copy(h3[64:128, 4, :, :, 0:13], h3[0:64, 4, :, :, 1:14])

            # ==================== Layer 4 ====================
            # conv4 64->1 + sigmoid: M=1, col-packed x4 at PE cols 0/32/64/96.
            # Tap-pair K-folding: rows 0:64 hold h3, rows 64:128 its g+1 shift,
            # so one K=128 matmul covers taps (dg=0, dg=1); dg=2 singles use
            # zeroed top-half weights (shifted col 13 is 0, so top adds 0).
            w4sb = wpool.tile([128, 54], BF16, tag="wb")
            nc.sync.dma_start(w4sb[:], w4t[:])
            y4 = stats.tile([1, 3, 4, 3, 12, 12], BF16, tag="hraw3")
            for d1o in range(3):
                for d2g in range(N_D2G):
                    psA = psum.tile([128, 3, 12, 12], F32, tag="ps")
                    psB = psum.tile([128, 3, 12, 12], F32, tag="ps")
                    for i in range(27):
                        dd, de, df = i // 9, (i // 3) % 3, i % 3
                        col = i % 4
                        for half in range(2):  # 0: dg=(0,1) pair, 1: dg=2
                            j = i + 27 * half
                            dgs = 0 if half == 0 else 2
                            rhs = h3[0:128, d1o + dd,
                                     3 * d2g + de:3 * d2g + de + 3,
                                     df:df + 12, dgs:dgs + 12]
                            pst = psA if half == 0 else psB
                            nc.tensor.matmul(
                                pst[32 * col:32 * col + 1, :],
                                w4sb[:, j:j + 1], rhs,
                                start=(i < 4), stop=(i >= 23),
                                tile_position=(0, 32 * col),
                            )
                    u1 = small.tile([1, 3, 12, 12], F32, tag="u1")
                    nc.scalar.copy(u1[:], psA[0:1, :])
                    for pst, pj in ((psA, 32), (psA, 64), (psA, 96),
                                    (psB, 0), (psB, 32), (psB, 64), (psB, 96)):
                        nc.vector.tensor_tensor(
                            out=u1[:], in0=u1[:], in1=pst[pj:pj + 1, :], op=ALU.add
                        )
                    nc.scalar.activation(
                        y4[:, d1o, d2g], u1[:], AF.Sigmoid, bias=b4sb[:]
                    )
            # y4 [1, d1o, d2g, 3, 12, 12] -> yout [1, 3, 4, 3, 12, 12]
            nc.sync.dma_start(yout.ap(), y4[:])

    nc.compile()
    return nc


_CACHE = {}
_BF = ml_dtypes.bfloat16


def _prep_static(w1, w2, w3, w4, g1, be1, g2, be2, g3, be3, b4):
    """Per-input-name concat arrays [8*dim0, ...] for everything except xps."""
    w1t = np.ascontiguousarray(
        np.transpose(np.asarray(w1, np.float32)[:, 0], (1, 2, 3, 4, 0))
    ).reshape(27, 3, 64)
    wt2 = np.transpose(np.asarray(w2, np.float32), (1, 2, 3, 4, 5, 0)).reshape(
        64, 81, 128
    )
    w2a = np.ascontiguousarray(np.concatenate([wt2, wt2], axis=0))  # [128,81,128]
    w3t = np.ascontiguousarray(
        np.transpose(np.asarray(w3, np.float32), (1, 2, 3, 4, 5, 0)).reshape(
            128, 81, 64
        )
    )
    # w4 pair layout [128, 54]: col j<27 = taps (3j+0 | 3j+1) stacked on K;
    # col 27+j = tap 3j+2 in rows 0:64 with zeroed top half.
    w4f = np.asarray(w4, np.float32)[0].reshape(64, 81)
    w4t = np.zeros((128, 54), np.float32)
    for jj in range(27):
        w4t[0:64, jj] = w4f[:, 3 * jj + 0]
        w4t[64:128, jj] = w4f[:, 3 * jj + 1]
        w4t[0:64, 27 + jj] = w4f[:, 3 * jj + 2]

    def rep(a):
        return np.ascontiguousarray(
            np.broadcast_to(a[None], (N_CORES, *a.shape)).reshape(
                N_CORES * a.shape[0], *a.shape[1:]
            )
        )

    out = {
        "w1t": rep(w1t.astype(_BF)), "w2a": rep(w2a.astype(_BF)),
        "w3t": rep(w3t.astype(_BF)), "w4t": rep(w4t.astype(_BF)),
        "g1": rep(np.asarray(g1, np.float32).reshape(64, 1)),
        "be1": rep(np.asarray(be1, np.float32).reshape(64, 1)),
        "g2": rep(np.asarray(g2, np.float32).reshape(128, 1)),
        "be2": rep(np.asarray(be2, np.float32).reshape(128, 1)),
        "g3": rep(np.asarray(g3, np.float32).reshape(64, 1)),
        "be3": rep(np.asarray(be3, np.float32).reshape(64, 1)),
        "b4": rep(np.asarray(b4, np.float32).reshape(1, 1)),
    }
    # per-core constants: masks + halo gather indices
    mlv = np.zeros((N_CORES, 1, 1), np.float32)
    mrv = np.zeros((N_CORES, 1, 1), np.float32)
    hidx128 = np.zeros((N_CORES, 128, 2), np.int32)
    hidx64 = np.zeros((N_CORES, 64, 2), np.int32)
    for c in range(N_CORES):
        q = c % 4
        mlv[c] = 0.0 if q == 0 else 1.0
        mrv[c] = 0.0 if q == 3 else 1.0
        ql, qr = (q - 1) % 4, (q + 1) % 4
        hidx128[c, :, 0] = (2 * ql + 1) * 128 + np.arange(128)
        hidx128[c, :, 1] = (2 * qr + 0) * 128 + np.arange(128)
        hidx64[c, :, 0] = (2 * ql + 1) * 64 + np.arange(64)
        hidx64[c, :, 1] = (2 * qr + 0) * 64 + np.arange(64)
    out["ml"] = mlv.reshape(N_CORES * 1, 1)
    out["mr"] = mrv.reshape(N_CORES * 1, 1)
    out["hidx128"] = hidx128.reshape(N_CORES * 128, 2)
    out["hidx64"] = hidx64.reshape(N_CORES * 64, 2)
    return out


def _get_state():
    if "st" in _CACHE:
        return _CACHE["st"]
    import jax
    from jax.sharding import Mesh, PartitionSpec, NamedSharding
    from jax.experimental.shard_map import shard_map
    from concourse import bass2jax

    nc = _build_module()
    bass2jax.install_neuronx_cc_hook()
    partition_name = nc.partition_id_tensor.name if nc.partition_id_tensor else None
    in_names, out_names, out_avals, zero_outs = [], [], [], []
    for alloc in nc.m.functions[0].allocations:
        if not isinstance(alloc, mybir.MemoryLocationSet):
            continue
        name = alloc.memorylocations[0].name
        if alloc.kind == "ExternalInput":
            if name != partition_name:
                in_names.append(name)
        elif alloc.kind == "ExternalOutput":
            shape = tuple(alloc.tensor_shape)
            dtype = mybir.dt.np(alloc.dtype)
            out_names.append(name)
            out_avals.append(jax.core.ShapedArray(shape, dtype))
            zero_outs.append(np.zeros(shape, dtype))
    n_params, n_outs = len(in_names), len(out_avals)
    bind_names = tuple(
        in_names + out_names + ([partition_name] if partition_name else []))

    def _body(*args):
        operands = list(args)
        if partition_name is not None:
            operands.append(bass2jax.partition_id_tensor())
        outs = bass2jax._bass_exec_p.bind(
            *operands, out_avals=tuple(out_avals), in_names=bind_names,
            out_names=tuple(out_names), lowering_input_output_aliases=(),
            sim_require_finite=True, sim_require_nnan=True, nc=nc)
        return tuple(outs)

    mesh = Mesh(np.asarray(jax.devices()[:N_CORES]), ("core",))
    sharding = NamedSharding(mesh, PartitionSpec("core"))
    in_specs = (PartitionSpec("core"),) * (n_params + n_outs)
    out_specs = (PartitionSpec("core"),) * n_outs

    # example args (zeros) — compile is weight-independent
    example_in = []
    for alloc in nc.m.functions[0].allocations:
        if not isinstance(alloc, mybir.MemoryLocationSet):
            continue
        if alloc.kind == "ExternalInput":
            name = alloc.memorylocations[0].name
            if name == partition_name:
                continue
            shape = tuple(alloc.tensor_shape)
            example_in.append(
                np.zeros((N_CORES * shape[0], *shape[1:]), mybir.dt.np(alloc.dtype))
            )
    concat_zeros = [
        np.zeros((N_CORES * z.shape[0], *z.shape[1:]), z.dtype) for z in zero_outs
    ]
    # No donation: yout is fully written by the kernel, so the pre-zeroed
    # output operand's content is never observed and can stay device-resident.
    try:
        compiled = bass2jax.fast_dispatch_compile(
            lambda: jax.jit(
                shard_map(_body, mesh=mesh, in_specs=in_specs,
                          out_specs=out_specs, check_rep=False),
                keep_unused=True,
            ).lower(*example_in, *concat_zeros).compile()
        )
        resident_zeros = [jax.device_put(z, sharding) for z in concat_zeros]
        jax.block_until_ready(resident_zeros)
    except Exception:
        compiled, resident_zeros = None, None
    _CACHE["concat_zeros_np"] = concat_zeros
    st = {
        "jax": jax,
        "nc": nc,
        "in_names": in_names,
        "compiled": compiled,
        "sharding": sharding,
        "resident_zeros": resident_zeros,
        "static_ids": None,
        "static_np": None,
        "static_dev": None,
        "static_concat": None,
    }
    _CACHE["st"] = st
    return st


def _ensure_static(st, statics):
    """statics: tuple of the 11 weight/param arrays in fixed order."""
    ids = tuple(id(a) for a in statics)
    if st["static_ids"] == ids:
        return
    if st["static_np"] is not None and all(
        np.array_equal(a, b) for a, b in zip(st["static_np"], statics)
    ):
        st["static_ids"] = ids
        return
    concat = _prep_static(*statics)
    st["static_concat"] = concat
    if st["compiled"] is not None:
        dev = {k: st["jax"].device_put(v, st["sharding"]) for k, v in concat.items()}
        st["jax"].block_until_ready(list(dev.values()))
        st["static_dev"] = dev
    st["static_np"] = tuple(np.asarray(a) for a in statics)
    st["static_ids"] = ids


def _run_fallback(st, xps):
    """Per-call dispatch through run_bass_kernel_spmd (AOT compile failed)."""
    from concourse.bass_utils import run_bass_kernel_spmd

    concat = st["static_concat"]
    in_maps = []
    for c in range(N_CORES):
        m = {}
        for nm, arr in concat.items():
            d0 = arr.shape[0] // N_CORES
            m[nm] = arr[c * d0:(c + 1) * d0]
        m["xps"] = xps[c:c + 1]
        in_maps.append(m)
    res = run_bass_kernel_spmd(st["nc"], in_maps, core_ids=list(range(N_CORES)))
    return np.stack([r["yout"][0] for r in res.results])


def kernel(x, w1, b1, g1, be1, w2, b2, g2, be2, w3, b3, g3, be3, w4, b4):
    # b1/b2/b3 cancel inside training-mode BN; b4 is applied before sigmoid.
    statics = (w1, w2, w3, w4, g1, be1, g2, be2, g3, be3, b4)

    # compact padded x shards: d1 pad 2 (margin windows reach d1 in [-2,13]),
    # e/f/g pad 1. Core (n,q) gets padded-d1 slices [3q, 3q+7).
    x = np.asarray(x, np.float32)
    xp = np.pad(x[:, 0], ((0, 0), (2, 2), (1, 1), (1, 1), (1, 1)))
    xps = np.empty((N_CORES, 7, 14, 14, 14), np.float32)
    for c in range(N_CORES):
        n, q = c // 4, c % 4
        xps[c] = xp[n, 3 * q:3 * q + 7]
    xps = xps.astype(_BF)

    y = None
    for attempt in range(4):
        try:
            st = _get_state()
            _ensure_static(st, statics)
            if st["compiled"] is None:
                y = _run_fallback(st, xps)
            else:
                args = [xps if nm == "xps" else st["static_dev"][nm]
                        for nm in st["in_names"]]
                outs = st["compiled"](*args, *st["resident_zeros"])
                y = np.asarray(outs[0])
            break
        except Exception:
            if attempt == 0:
                continue  # transient: immediate retry
            if attempt >= 3:
                raise
            # axon worker may have crashed; it self-restarts in ~60s and all
            # device state (buffers, executable) dies with it. Full rebuild.
            import time as _time

            _time.sleep(75)
            try:
                from jax._src import xla_bridge as _xb

                _xb._clear_backends()
            except Exception:
                pass
            _CACHE.clear()

    y = y.astype(np.float32).reshape(N_CORES, 3, 4, 3, 12, 12)
    out = np.empty((2, 1, 12, 12, 12, 12), np.float32)
    for c in range(N_CORES):
        n, q = c // 4, c % 4
        out[n, 0, 3 * q:3 * q + 3] = y[c].reshape(3, 12, 12, 12)
    return out
